# revision 24
# baseline (speedup 1.0000x reference)
"""Trainium2 Bass kernel for nn_Attention_13984413516503 (sparse_attention).

Sharding: 16 heads tensor-parallel over 8 NeuronCores (2 heads/core).
Per core: QKV projections for its heads, RoPE, two-softmax gated attention,
AllGather of per-head attention outputs (bf16), sharded o_proj (each core
produces a 256-wide slice of the output features).

All shapes hardcoded for: B=2, S=1024, D=2048, H=16, HD=128, AL=10.
"""

import math

import numpy as np
import ml_dtypes

BF16 = ml_dtypes.bfloat16

B, S, D = 2, 1024, 2048
H, HD = 16, 128
AL = 10          # adapter length
MF = 10          # MAX_FEATS
NCORES = 8
HPC = H // NCORES          # heads per core = 2
TOK = B * S                # 2048
XTOK = AL + TOK            # 2058 (adapter ++ tokens)
ISC = 1.0 / math.sqrt(HD)  # 1/sqrt(128)

_BUILT = None   # (nc, names) cache
LAST_EXEC_NS = None


def _build():
    import concourse.bass as bass
    import concourse.mybir as mybir
    import concourse.tile as tile
    from concourse import bacc

    dt = mybir.dt
    AF = mybir.ActivationFunctionType

    nc = bacc.Bacc(
        "TRN2", target_bir_lowering=False, debug=False, num_devices=NCORES
    )

    # ---- kernel I/O ----
    xa = nc.dram_tensor("xa", [D, XTOK], dt.bfloat16, kind="ExternalInput")
    wqkv = nc.dram_tensor("wqkv", [D, 6 * HD], dt.bfloat16, kind="ExternalInput")
    wo = nc.dram_tensor("wo", [D, HPC * HD], dt.bfloat16, kind="ExternalInput")
    c2d = nc.dram_tensor("c2", [HD, TOK], dt.bfloat16, kind="ExternalInput")
    s2d = nc.dram_tensor("s2", [HD, TOK], dt.bfloat16, kind="ExternalInput")
    trid = nc.dram_tensor("tri", [HD, HD], dt.bfloat16, kind="ExternalInput")
    identd = nc.dram_tensor("ident", [HD, HD], dt.bfloat16, kind="ExternalInput")
    g2md = nc.dram_tensor("g2m", [HD, HPC * S], dt.bfloat16, kind="ExternalInput")
    browd = nc.dram_tensor("brow", [1, 3 * HD], dt.bfloat16, kind="ExternalInput")
    out_ext = nc.dram_tensor("out", [HPC * HD, TOK], dt.float32, kind="ExternalOutput")

    # internal DRAM for the collective
    wupin = nc.dram_tensor("wupin", [8, 64], dt.bfloat16)
    wupout = nc.dram_tensor("wupout", [64, 64], dt.bfloat16, addr_space="Shared")
    bnc = [nc.dram_tensor(f"agin{b}", [HPC * HD, S], dt.bfloat16) for b in range(B)]
    agd = [
        nc.dram_tensor(f"agout{b}", [H * HD, S], dt.bfloat16, addr_space="Shared")
        for b in range(B)
    ]
    RG = [list(range(NCORES))]

    KT16 = D // 128  # 16 contraction tiles for projections / o_proj

    with tile.TileContext(nc, num_cores=NCORES) as tc:
        import contextlib

        ctx = contextlib.ExitStack()
        with ctx:
            psum3 = ctx.enter_context(tc.tile_pool(name="psum3", bufs=3, space="PSUM"))
            psum = ctx.enter_context(tc.tile_pool(name="psum", bufs=2, space="PSUM"))
            psum1 = ctx.enter_context(tc.tile_pool(name="psum1", bufs=1, space="PSUM"))
            consts = ctx.enter_context(tc.tile_pool(name="consts", bufs=1))
            work = ctx.enter_context(tc.tile_pool(name="work", bufs=1))

            # ---- persistent constants (DMAs issued later, after critical loads) ----
            wo_sb = consts.tile([128, KT16, HPC * HD], dt.bfloat16, tag="wo")
            c2 = consts.tile([HD, TOK], dt.bfloat16, tag="c2")
            s2 = consts.tile([HD, TOK], dt.bfloat16, tag="s2")
            tri = consts.tile([HD, HD], dt.bfloat16, tag="tri")
            ident = consts.tile([HD, HD], dt.bfloat16, tag="ident")
            g2m = consts.tile([HD, HPC * S], dt.bfloat16, tag="g2m")
            brow = consts.tile([1, 3 * HD], dt.bfloat16, tag="brow")
            ocol = consts.tile([128, 1], dt.bfloat16, tag="ocol")
            nc.vector.memset(ocol[:], 1.0)


            # proj destinations: QR, QI, KR, KI, V0, V1  (paired-head layout)
            pdst = [
                work.tile([128, XTOK], dt.bfloat16, tag=f"pd{m}", name=f"pd{m}") for m in range(6)
            ]

            # ---- phases 1-3: chunk-pipelined projections + RoPE + V-transpose ----
            # m-tiles: 0=QR 1=QI 2=KR 3=KI 4=V0 5=V1 (paired-head layout)
            QR, QI, KR, KI = pdst[0], pdst[1], pdst[2], pdst[3]
            VT = [pdst[4], pdst[5]]
            QT = [work.tile([128, XTOK], dt.bfloat16, tag=f"qt{h}", name=f"qt{h}") for h in range(HPC)]
            KTt = [work.tile([128, XTOK], dt.bfloat16, tag=f"kt{h}", name=f"kt{h}") for h in range(HPC)]
            vtr = [work.tile([128, B * 8, 128], dt.bfloat16, tag=f"vtr{h}", name=f"vtr{h}") for h in range(HPC)]
            avt = [work.tile([AL, 128], dt.bfloat16, tag=f"avt{h}", name=f"avt{h}") for h in range(HPC)]

            attnT = [work.tile([128, TOK], dt.bfloat16, tag=f"at{h}", name=f"at{h}") for h in range(HPC)]
            epool = ctx.enter_context(tc.tile_pool(name="epool", bufs=8))
            eapool = ctx.enter_context(tc.tile_pool(name="eapool", bufs=2))
            npool = ctx.enter_context(tc.tile_pool(name="npool", bufs=2))

            # ---- phase 4: attention (emitted per batch; b0 before last proj chunk) ----
            def attn_batch(b):
                for h in range(HPC):
                    base_k = AL + S * b
                    for qc in range(2):
                        qcol = base_k + 512 * qc
                        nt = 4 * qc + 4
                        # adapter scores -> Ea
                        sa = psum3.tile([128, 512], dt.float32, tag="mm")
                        nc.tensor.matmul(
                            sa[:AL, :], KTt[h][:, 0:AL], QT[h][:, qcol : qcol + 512],
                            start=True, stop=True,
                        )
                        ea = eapool.tile([AL, 512], dt.bfloat16, tag="ea")
                        nc.scalar.activation(ea[:], sa[:AL, :], AF.Exp, scale=ISC)
                        da = psum.tile([1, 512], dt.float32, tag="dd")
                        nc.tensor.matmul(da[:], ocol[0:AL, :], ea[:], start=True, stop=True)
                        # video scores -> Ev tiles
                        evs = []
                        for t in range(nt):
                            sp = psum3.tile([128, 512], dt.float32, tag="mm")
                            nc.tensor.matmul(
                                sp[:],
                                KTt[h][:, base_k + 128 * t : base_k + 128 * (t + 1)],
                                QT[h][:, qcol : qcol + 512],
                                start=True, stop=True,
                            )
                            ev = epool.tile([128, 512], dt.bfloat16, tag="ev")
                            nc.scalar.activation(ev[:], sp[:], AF.Exp, scale=ISC)
                            j = t - 4 * qc
                            if j >= 0:
                                if j > 0:
                                    nc.vector.memset(ev[:, 0 : 128 * j], 0.0)
                                nc.vector.tensor_mul(
                                    ev[:, 128 * j : 128 * (j + 1)],
                                    ev[:, 128 * j : 128 * (j + 1)],
                                    tri[:],
                                )
                            if t == 0:
                                nc.vector.tensor_mul(
                                    ev[:], ev[:], g2m[:, S * h + 512 * qc : S * h + 512 * (qc + 1)]
                                )
                            evs.append(ev)
                        # denominators
                        dv = psum.tile([1, 512], dt.float32, tag="dd")
                        for i, ev in enumerate(evs):
                            nc.tensor.matmul(
                                dv[:], ocol[:], ev[:], start=(i == 0), stop=(i == nt - 1)
                            )
                        # video PV accumulation
                        pv = psum.tile([128, 512], dt.float32, tag="pv")
                        for i, ev in enumerate(evs):
                            nc.tensor.matmul(
                                pv[:], vtr[h][:, 8 * b + i, :], ev[:],
                                start=(i == 0), stop=False, skip_group_check=True,
                            )
                        # adapter rescale: Ea' = Ea * (tanh(g1)*Dv/Da), then fold into pv
                        raf = npool.tile([1, 512], dt.float32, tag="nf")
                        nc.vector.reciprocal_approx_fast(raf[:], da[:])
                        rr = npool.tile([1, 512], dt.float32, tag="nf")
                        nc.vector.tensor_mul(rr[:], raf[:], dv[:])
                        rr16 = npool.tile([1, 512], dt.bfloat16, tag="n16")
                        nc.scalar.copy(rr16[:], rr[:])
                        eas = psum1.tile([128, 512], dt.float32, tag="bc")
                        nc.tensor.matmul(
                            eas[:AL, :],
                            brow[0:1, 128 * (1 + h) : 128 * (1 + h) + AL],
                            rr16[:], start=True, stop=True,
                        )
                        ea2 = eapool.tile([AL, 512], dt.bfloat16, tag="ea2")
                        nc.vector.tensor_mul(ea2[:], ea[:], eas[:AL, :])
                        nc.tensor.matmul(
                            pv[:], avt[h][:], ea2[:], start=False, stop=True,
                            skip_group_check=True,
                        )
                        # normalize by 1/Dv and store attnT slice
                        rvf = npool.tile([1, 512], dt.float32, tag="nf")
                        nc.vector.reciprocal_approx_fast(rvf[:], dv[:])
                        rv16 = npool.tile([1, 512], dt.bfloat16, tag="n16")
                        nc.scalar.copy(rv16[:], rvf[:])
                        rvb_ps = psum1.tile([128, 512], dt.float32, tag="bc")
                        nc.tensor.matmul(
                            rvb_ps[:], brow[0:1, 0:128], rv16[:], start=True, stop=True
                        )
                        rvb = npool.tile([128, 512], dt.bfloat16, tag="rvb")
                        nc.scalar.copy(rvb[:], rvb_ps[:])
                        nc.vector.tensor_mul(
                            attnT[h][:, S * b + 512 * qc : S * b + 512 * (qc + 1)],
                            pv[:], rvb[:],
                        )
                # after batch b: bounce + AllGather
                for h in range(HPC):
                    nc.sync.dma_start(
                        bnc[b][128 * h : 128 * (h + 1), :], attnT[h][:, S * b : S * (b + 1)]
                    )
                nc.gpsimd.collective_compute(
                    "AllGather",
                    bass.mybir.AluOpType.bypass,
                    replica_groups=RG,
                    ins=[bnc[b][:, :].opt()],
                    outs=[agd[b][:, :].opt()],
                )



            # warmup collective: absorb ncfw/channel startup cost during load
            nc.gpsimd.collective_compute(
                "AllGather", bass.mybir.AluOpType.bypass, replica_groups=RG,
                ins=[wupin[:, :].opt()], outs=[wupout[:, :].opt()],
            )

            with tc.tile_pool(name="p1", bufs=1) as p1pool, tc.tile_pool(name="rope", bufs=2) as rp:
                wq_k = [p1pool.tile([128, 6 * HD], dt.bfloat16, tag=f"wq{k}", name=f"wq{k}") for k in range(KT16)]
                # xa tiles per (cchunk, k): col ranges [0:522),[522:1034),[1034:1546),[1546:2058)
                ccol = [(0, 522), (522, 512), (1034, 512), (1546, 512)]
                xs = [
                    [p1pool.tile([128, 522], dt.bfloat16, tag=f"xa{min(ci,3) if ci < 3 else 0}_{k}", name=f"xa{ci}_{k}") for k in range(KT16)]
                    for ci in range(4)
                ]
                for k in range(KT16):
                    nc.sync.dma_start(wq_k[k][:, 256:512], wqkv[128 * k : 128 * (k + 1), 256:512])
                    x0, xw = ccol[0]
                    nc.sync.dma_start(xs[0][k][:, :xw], xa[128 * k : 128 * (k + 1), x0 : x0 + xw])
                for k in range(KT16):
                    nc.sync.dma_start(wq_k[k][:, 0:256], wqkv[128 * k : 128 * (k + 1), 0:256])
                    nc.sync.dma_start(wq_k[k][:, 512:768], wqkv[128 * k : 128 * (k + 1), 512:768])
                # consts after the chunk-0-critical loads, before the rest of xa
                nc.sync.dma_start(c2[:], c2d[:, :])
                nc.sync.dma_start(s2[:], s2d[:, :])
                nc.sync.dma_start(ident[:], identd[:, :])
                nc.sync.dma_start(tri[:], trid[:, :])
                nc.sync.dma_start(g2m[:], g2md[:, :])
                nc.sync.dma_start(brow[:], browd[:, :])
                nc.sync.dma_start(wo_sb[:], wo[:, :].rearrange("(k p) c -> p k c", p=128))
                for ci in range(1, 4):
                    x0, xw = ccol[ci]
                    for k in range(KT16):
                        nc.sync.dma_start(xs[ci][k][:, :xw], xa[128 * k : 128 * (k + 1), x0 : x0 + xw])


                def proj_group(m, c0, w, row, xoff):
                    ps = psum3.tile([128, 512], dt.float32, tag="mm")
                    for k in range(KT16):
                        nc.tensor.matmul(
                            ps[:, :w],
                            wq_k[k][:, 128 * m : 128 * (m + 1)],
                            row[k][:, xoff : xoff + w],
                            start=(k == 0), stop=(k == KT16 - 1),
                        )
                    nc.scalar.copy(pdst[m][:, c0 : c0 + w], ps[:, :w])


                def rope_chunk(xr, xi, tc0, c0):
                    # tc0: token col offset in [0,2048); c0 = AL + tc0 (col in pdst)
                    cs = c2[:, tc0 : tc0 + 512]
                    sn = s2[:, tc0 : tc0 + 512]
                    a = rp.tile([128, 512], dt.bfloat16, tag="ra")
                    b_ = rp.tile([128, 512], dt.bfloat16, tag="rb")
                    nc.vector.tensor_mul(a[:], xr[:, c0 : c0 + 512], cs)
                    nc.vector.tensor_mul(b_[:], xi[:, c0 : c0 + 512], sn)
                    ro = rp.tile([128, 512], dt.bfloat16, tag="rro")
                    nc.vector.tensor_sub(ro[:], a[:], b_[:])
                    c_ = rp.tile([128, 512], dt.bfloat16, tag="rc")
                    d_ = rp.tile([128, 512], dt.bfloat16, tag="rd")
                    nc.vector.tensor_mul(c_[:], xr[:, c0 : c0 + 512], sn)
                    nc.vector.tensor_mul(d_[:], xi[:, c0 : c0 + 512], cs)
                    io = rp.tile([128, 512], dt.bfloat16, tag="rio")
                    nc.vector.tensor_add(io[:], c_[:], d_[:])
                    return ro, io

                def post_m(m, ci):
                    c0 = AL + 512 * ci
                    tc0 = 512 * ci
                    if m == 3:   # KR+KI done for this chunk
                        ro, io = rope_chunk(KR, KI, tc0, c0)
                        for h in range(HPC):
                            hs = slice(64 * h, 64 * h + 64)
                            nc.sync.dma_start(KTt[h][0:64, c0 : c0 + 512], ro[hs, :])
                            nc.sync.dma_start(KTt[h][64:128, c0 : c0 + 512], io[hs, :])
                    elif m == 1:  # QR+QI done
                        ro, io = rope_chunk(QR, QI, tc0, c0)
                        for h in range(HPC):
                            hs = slice(64 * h, 64 * h + 64)
                            nc.sync.dma_start(QT[h][0:64, c0 : c0 + 512], ro[hs, :])
                            nc.sync.dma_start(QT[h][64:128, c0 : c0 + 512], io[hs, :])
                    elif m >= 4:  # V chunk ready -> transposes
                        h = m - 4
                        bb, thalf = ci // 2, 4 * (ci % 2)
                        for tt in range(4):
                            tp = psum.tile([128, 128], dt.bfloat16, tag="pv")
                            nc.tensor.transpose(tp[:], VT[h][:, c0 + 128 * tt : c0 + 128 * (tt + 1)], ident[:])
                            nc.scalar.copy(vtr[h][:, 8 * bb + thalf + tt, :], tp[:])

                # chunk 0 solo (starts as soon as xs[0] lands), then pairs (1,2), then 3
                for m in (2, 3, 0, 1, 4, 5):
                    psa = psum3.tile([128, 512], dt.float32, tag="mm")
                    for k in range(KT16):
                        nc.tensor.matmul(psa[:], wq_k[k][:, 128 * m : 128 * (m + 1)],
                                         xs[0][k][:, AL : AL + 512],
                                         start=(k == 0), stop=(k == KT16 - 1))
                    nc.scalar.copy(pdst[m][:, AL : AL + 512], psa[:])
                    post_m(m, 0)
                # adapter column groups (K and V only) — emitted after chunk 0 so the
                # LDW-bound tiny matmuls run on a warm PE instead of gating the start
                for m in (2, 3, 4, 5):
                    proj_group(m, 0, AL, xs[0], 0)
                for h in range(HPC):
                    hs = slice(64 * h, 64 * h + 64)
                    nc.sync.dma_start(KTt[h][0:64, 0:AL], KR[hs, 0:AL])
                    nc.sync.dma_start(KTt[h][64:128, 0:AL], KI[hs, 0:AL])
                    tp = psum.tile([128, 128], dt.bfloat16, tag="pv")
                    nc.tensor.transpose(tp[:AL, :], VT[h][:, 0:AL], ident[:])
                    nc.scalar.copy(avt[h][:], tp[:AL, :])
                for ca, cb in ((1, 2),):
                    for m in (2, 3, 0, 1, 4, 5):
                        psa = psum3.tile([128, 512], dt.float32, tag="mm")
                        psb = psum3.tile([128, 512], dt.float32, tag="mm")
                        for k in range(KT16):
                            lhs = wq_k[k][:, 128 * m : 128 * (m + 1)]
                            nc.tensor.matmul(psa[:], lhs, xs[ca][k][:, 0:512],
                                             start=(k == 0), stop=(k == KT16 - 1))
                            nc.tensor.matmul(psb[:], lhs, xs[cb][k][:, 0:512],
                                             start=(k == 0), stop=(k == KT16 - 1))
                        nc.scalar.copy(pdst[m][:, AL + 512 * ca : AL + 512 * (ca + 1)], psa[:])
                        nc.scalar.copy(pdst[m][:, AL + 512 * cb : AL + 512 * (cb + 1)], psb[:])
                        post_m(m, ca)
                        post_m(m, cb)
                attn_batch(0)
                for m in (2, 3, 0, 1, 4, 5):
                    psa = psum3.tile([128, 512], dt.float32, tag="mm")
                    for k in range(KT16):
                        nc.tensor.matmul(psa[:], wq_k[k][:, 128 * m : 128 * (m + 1)],
                                         xs[3][k][:, 0:512],
                                         start=(k == 0), stop=(k == KT16 - 1))
                    nc.scalar.copy(pdst[m][:, AL + 512 * 3 : AL + 512 * 4], psa[:])
                    post_m(m, 3)
                attn_batch(1)

            # ---- phase 5: o_proj on gathered heads ----
            ogp = ctx.enter_context(tc.tile_pool(name="ogp", bufs=2))
            for b in range(B):
                ag_k = [ogp.tile([128, S], dt.bfloat16, tag=f"ag{k}", name=f"ag{k}") for k in range(KT16)]
                for k in range(KT16):
                    nc.sync.dma_start(ag_k[k][:], agd[b][128 * k : 128 * (k + 1), :])
                for j in range(HPC):
                    pa = psum3.tile([128, 512], dt.float32, tag="mm")
                    pb = psum3.tile([128, 512], dt.float32, tag="mm")
                    for k in range(KT16):
                        lhs = wo_sb[:, k, 128 * j : 128 * (j + 1)]
                        nc.tensor.matmul(pa[:], lhs, ag_k[k][:, 0:512],
                                         start=(k == 0), stop=(k == KT16 - 1))
                        nc.tensor.matmul(pb[:], lhs, ag_k[k][:, 512:1024],
                                         start=(k == 0), stop=(k == KT16 - 1))
                    for qc, ps in ((0, pa), (1, pb)):
                        osb = ogp.tile([128, 512], dt.float32, tag="osb")
                        nc.scalar.copy(osb[:], ps[:])
                        nc.sync.dma_start(
                            out_ext[128 * j : 128 * (j + 1),
                                    S * b + 512 * qc : S * b + 512 * (qc + 1)],
                            osb[:],
                        )

    nc.finalize()
    return nc


def _host_prep(inputs):
    """Build the 8 per-core input maps from full inputs."""
    x = np.asarray(inputs["x"], np.float32)
    adapter = np.asarray(inputs["adapter"], np.float32)
    wq = np.asarray(inputs["wq"], np.float32)
    wk = np.asarray(inputs["wk"], np.float32)
    wv = np.asarray(inputs["wv"], np.float32)
    wo = np.asarray(inputs["wo"], np.float32)
    g1 = np.asarray(inputs["gate1"], np.float32).reshape(H)
    g2 = np.asarray(inputs["gate2"], np.float32).reshape(H)
    fc = np.asarray(inputs["freqs_cos"], np.float32)  # [S, 64]
    fs = np.asarray(inputs["freqs_sin"], np.float32)
    vs = int(inputs["video_start"])
    assert vs + MF <= 128, "gate2 block must stay in kt tile 0"

    # xa: [D, 10+2048] = adapter^T ++ x^T (bf16)
    xt = x.reshape(TOK, D).T
    at = adapter.reshape(AL, D).T
    xa = np.concatenate([at, xt], axis=1).astype(BF16)

    # RoPE split permutation per head: even dims then odd dims
    ev = np.arange(0, HD, 2)
    od = np.arange(1, HD, 2)

    # c2/s2: [128, 2048]; rows 0-63 for head h0's pairs, 64-127 for h1's pairs
    cosT = np.tile(fc.T, (1, B))  # [64, 2048]
    sinT = np.tile(fs.T, (1, B))
    c2 = np.vstack([cosT, cosT]).astype(BF16)
    s2 = np.vstack([sinT, sinT]).astype(BF16)

    tri = np.triu(np.ones((HD, HD), np.float32)).astype(BF16)
    ident = np.eye(HD, dtype=np.float32).astype(BF16)

    in_maps = []
    for c in range(NCORES):
        hs = [HPC * c + i for i in range(HPC)]  # global head ids
        # paired-head m-tiles: QR=[h0_even,h1_even], QI=[h0_odd,h1_odd], same for K; V=[h0],[h1]
        def rows(w, h):  # weight rows for head h -> [128, D]
            return w[HD * h : HD * (h + 1), :]

        qr = np.vstack([rows(wq, hs[0])[ev], rows(wq, hs[1])[ev]])
        qi = np.vstack([rows(wq, hs[0])[od], rows(wq, hs[1])[od]])
        kr = np.vstack([rows(wk, hs[0])[ev], rows(wk, hs[1])[ev]])
        ki = np.vstack([rows(wk, hs[0])[od], rows(wk, hs[1])[od]])
        v0 = rows(wv, hs[0])
        v1 = rows(wv, hs[1])
        wqkv = np.concatenate([m.T for m in (qr, qi, kr, ki, v0, v1)], axis=1).astype(BF16)

        woc = wo.T[:, HPC * HD * c : HPC * HD * (c + 1)].astype(BF16)  # [D, 256]

        g2mat = np.ones((HD, HPC * S), np.float32)
        for i, h in enumerate(hs):
            blk = np.ones((HD, S), np.float32)
            blk[vs : vs + MF, vs + MF :] = math.exp(g2[h])
            g2mat[:, S * i : S * (i + 1)] = blk
        g2mat = g2mat.astype(BF16)

        brow = np.zeros((1, 3 * HD), np.float32)
        brow[0, 0:HD] = 1.0
        for i, h in enumerate(hs):
            brow[0, HD * (1 + i) : HD * (2 + i)] = math.tanh(g1[h])
        brow = brow.astype(BF16)

        in_maps.append(
            {
                "xa": xa, "wqkv": wqkv, "wo": woc, "c2": c2, "s2": s2,
                "tri": tri, "ident": ident, "g2m": g2mat, "brow": brow,
            }
        )
    return in_maps


def _enable_ldw_opt():
    """Walrus dedups consecutive identical LDWEIGHTS only with ldw-opt on;
    our chunk-paired accumulation groups repeat each stationary operand."""
    from concourse import bass_utils
    if getattr(bass_utils, "_ldw_patched", False):
        return
    orig = bass_utils.run_command

    def patched(cmd, *a, **kw):
        if isinstance(cmd, list):
            cmd = [c.replace("--enable-ldw-opt=false", "--enable-ldw-opt=true") if isinstance(c, str) else c for c in cmd]
        return orig(cmd, *a, **kw)

    bass_utils.run_command = patched
    bass_utils._ldw_patched = True


def _ensure_ntff_hook():
    import sys, types
    if "antenv.axon_hooks" in sys.modules:
        return
    try:
        from trn_agent_boot.trn_boot import _ntff_profile_via_ctypes
        hook = _ntff_profile_via_ctypes("/opt/axon/libaxon_pjrt.so")
        mod = types.ModuleType("antenv.axon_hooks")
        mod.get_axon_ntff_profile_hook = lambda: hook
        mod.set_axon_ntff_profile_hook = lambda h: None
        sys.modules["antenv.axon_hooks"] = mod
    except Exception:
        pass


def kernel(**inputs):
    global _BUILT, LAST_EXEC_NS
    import os
    from concourse.bass_utils import run_bass_kernel_spmd

    if _BUILT is None:
        _BUILT = _build()
    nc = _BUILT
    in_maps = _host_prep(inputs)
    trace = bool(os.environ.get("KERNEL_TRACE"))
    if trace:
        _ensure_ntff_hook()
    res = run_bass_kernel_spmd(
        nc, in_maps, core_ids=list(range(NCORES)), trace=trace
    )
    LAST_EXEC_NS = res.exec_time_ns
    outs = [np.asarray(r["out"], np.float32) for r in res.results]
    # out_c: [256, 2048] = out^T[j_local, b*1024+s] -> full [B, S, D]
    full = np.concatenate(
        [o.reshape(HPC * HD, B, S).transpose(1, 2, 0) for o in outs], axis=2
    )
    return full.astype(np.float32)


# revision 25
# speedup vs baseline: 1.0030x; 1.0030x over previous
"""Trainium2 Bass kernel for nn_Attention_13984413516503 (sparse_attention).

Sharding: 16 heads tensor-parallel over 8 NeuronCores (2 heads/core).
Per core: QKV projections for its heads, RoPE, two-softmax gated attention,
AllGather of per-head attention outputs (bf16), sharded o_proj (each core
produces a 256-wide slice of the output features).

All shapes hardcoded for: B=2, S=1024, D=2048, H=16, HD=128, AL=10.
"""

import math

import numpy as np
import ml_dtypes

BF16 = ml_dtypes.bfloat16

B, S, D = 2, 1024, 2048
H, HD = 16, 128
AL = 10          # adapter length
MF = 10          # MAX_FEATS
NCORES = 8
HPC = H // NCORES          # heads per core = 2
TOK = B * S                # 2048
XTOK = AL + TOK            # 2058 (adapter ++ tokens)
ISC = 1.0 / math.sqrt(HD)  # 1/sqrt(128)

_BUILT = None   # (nc, names) cache
LAST_EXEC_NS = None


def _build():
    import concourse.bass as bass
    import concourse.mybir as mybir
    import concourse.tile as tile
    from concourse import bacc

    dt = mybir.dt
    AF = mybir.ActivationFunctionType

    nc = bacc.Bacc(
        "TRN2", target_bir_lowering=False, debug=False, num_devices=NCORES
    )

    # ---- kernel I/O ----
    xa = nc.dram_tensor("xa", [D, XTOK], dt.bfloat16, kind="ExternalInput")
    wqkv = nc.dram_tensor("wqkv", [D, 6 * HD], dt.bfloat16, kind="ExternalInput")
    wo = nc.dram_tensor("wo", [D, HPC * HD], dt.bfloat16, kind="ExternalInput")
    c2d = nc.dram_tensor("c2", [HD, TOK], dt.bfloat16, kind="ExternalInput")
    s2d = nc.dram_tensor("s2", [HD, TOK], dt.bfloat16, kind="ExternalInput")
    trid = nc.dram_tensor("tri", [HD, HD], dt.bfloat16, kind="ExternalInput")
    identd = nc.dram_tensor("ident", [HD, HD], dt.bfloat16, kind="ExternalInput")
    g2md = nc.dram_tensor("g2m", [HD, HPC * S], dt.bfloat16, kind="ExternalInput")
    browd = nc.dram_tensor("brow", [1, 3 * HD], dt.bfloat16, kind="ExternalInput")
    out_ext = nc.dram_tensor("out", [HPC * HD, TOK], dt.float32, kind="ExternalOutput")

    # internal DRAM for the collective
    wupin = nc.dram_tensor("wupin", [8, 64], dt.bfloat16)
    wupout = nc.dram_tensor("wupout", [64, 64], dt.bfloat16, addr_space="Shared")
    bnc = [nc.dram_tensor(f"agin{b}", [HPC * HD, S], dt.bfloat16) for b in range(B)]
    agd = [
        nc.dram_tensor(f"agout{b}", [H * HD, S], dt.bfloat16, addr_space="Shared")
        for b in range(B)
    ]
    RG = [list(range(NCORES))]

    KT16 = D // 128  # 16 contraction tiles for projections / o_proj

    with tile.TileContext(nc, num_cores=NCORES) as tc:
        import contextlib

        ctx = contextlib.ExitStack()
        with ctx:
            psum3 = ctx.enter_context(tc.tile_pool(name="psum3", bufs=3, space="PSUM"))
            psum = ctx.enter_context(tc.tile_pool(name="psum", bufs=2, space="PSUM"))
            psum1 = ctx.enter_context(tc.tile_pool(name="psum1", bufs=1, space="PSUM"))
            consts = ctx.enter_context(tc.tile_pool(name="consts", bufs=1))
            work = ctx.enter_context(tc.tile_pool(name="work", bufs=1))

            # ---- persistent constants (DMAs issued later, after critical loads) ----
            wo_sb = consts.tile([128, KT16, HPC * HD], dt.bfloat16, tag="wo")
            c2 = consts.tile([HD, TOK], dt.bfloat16, tag="c2")
            s2 = consts.tile([HD, TOK], dt.bfloat16, tag="s2")
            tri = consts.tile([HD, HD], dt.bfloat16, tag="tri")
            ident = consts.tile([HD, HD], dt.bfloat16, tag="ident")
            g2m = consts.tile([HD, HPC * S], dt.bfloat16, tag="g2m")
            brow = consts.tile([1, 3 * HD], dt.bfloat16, tag="brow")
            ocol = consts.tile([128, 1], dt.bfloat16, tag="ocol")
            nc.vector.memset(ocol[:], 1.0)


            # proj destinations: QR, QI, KR, KI, V0, V1  (paired-head layout)
            pdst = [
                work.tile([128, XTOK], dt.bfloat16, tag=f"pd{m}", name=f"pd{m}") for m in range(6)
            ]

            # ---- phases 1-3: chunk-pipelined projections + RoPE + V-transpose ----
            # m-tiles: 0=QR 1=QI 2=KR 3=KI 4=V0 5=V1 (paired-head layout)
            QR, QI, KR, KI = pdst[0], pdst[1], pdst[2], pdst[3]
            VT = [pdst[4], pdst[5]]
            QT = [work.tile([128, XTOK], dt.bfloat16, tag=f"qt{h}", name=f"qt{h}") for h in range(HPC)]
            KTt = [work.tile([128, XTOK], dt.bfloat16, tag=f"kt{h}", name=f"kt{h}") for h in range(HPC)]
            vtr = [work.tile([128, B * 8, 128], dt.bfloat16, tag=f"vtr{h}", name=f"vtr{h}") for h in range(HPC)]
            avt = [work.tile([AL, 128], dt.bfloat16, tag=f"avt{h}", name=f"avt{h}") for h in range(HPC)]

            attnT = [work.tile([128, TOK], dt.bfloat16, tag=f"at{h}", name=f"at{h}") for h in range(HPC)]
            epool = ctx.enter_context(tc.tile_pool(name="epool", bufs=8))
            eapool = ctx.enter_context(tc.tile_pool(name="eapool", bufs=2))
            npool = ctx.enter_context(tc.tile_pool(name="npool", bufs=2))

            # ---- phase 4: attention (emitted per batch; b0 before last proj chunk) ----
            def attn_batch(b):
                for h in range(HPC):
                    base_k = AL + S * b
                    for qc in range(2):
                        qcol = base_k + 512 * qc
                        nt = 4 * qc + 4
                        # adapter scores -> Ea
                        sa = psum3.tile([128, 512], dt.float32, tag="mm")
                        nc.tensor.matmul(
                            sa[:AL, :], KTt[h][:, 0:AL], QT[h][:, qcol : qcol + 512],
                            start=True, stop=True,
                        )
                        ea = eapool.tile([AL, 512], dt.bfloat16, tag="ea")
                        nc.scalar.activation(ea[:], sa[:AL, :], AF.Exp, scale=ISC)
                        # video scores -> Ev tiles
                        evs = []
                        for t in range(nt):
                            sp = psum3.tile([128, 512], dt.float32, tag="mm")
                            nc.tensor.matmul(
                                sp[:],
                                KTt[h][:, base_k + 128 * t : base_k + 128 * (t + 1)],
                                QT[h][:, qcol : qcol + 512],
                                start=True, stop=True,
                            )
                            ev = epool.tile([128, 512], dt.bfloat16, tag="ev")
                            nc.scalar.activation(ev[:], sp[:], AF.Exp, scale=ISC)
                            j = t - 4 * qc
                            if j >= 0:
                                if j > 0:
                                    nc.vector.memset(ev[:, 0 : 128 * j], 0.0)
                                nc.vector.tensor_mul(
                                    ev[:, 128 * j : 128 * (j + 1)],
                                    ev[:, 128 * j : 128 * (j + 1)],
                                    tri[:],
                                )
                            if t == 0:
                                nc.vector.tensor_mul(
                                    ev[:], ev[:], g2m[:, S * h + 512 * qc : S * h + 512 * (qc + 1)]
                                )
                            evs.append(ev)
                        # denominators (da emitted here so its psum-slot wait
                        # doesn't stall the in-order PE stream before the scores)
                        da = psum.tile([1, 512], dt.float32, tag="dd")
                        nc.tensor.matmul(da[:], ocol[0:AL, :], ea[:], start=True, stop=True)
                        dv = psum.tile([1, 512], dt.float32, tag="dd")
                        for i, ev in enumerate(evs):
                            nc.tensor.matmul(
                                dv[:], ocol[:], ev[:], start=(i == 0), stop=(i == nt - 1)
                            )
                        # video PV accumulation
                        pv = psum.tile([128, 512], dt.float32, tag="pv")
                        for i, ev in enumerate(evs):
                            nc.tensor.matmul(
                                pv[:], vtr[h][:, 8 * b + i, :], ev[:],
                                start=(i == 0), stop=False, skip_group_check=True,
                            )
                        # adapter rescale: Ea' = Ea * (tanh(g1)*Dv/Da), then fold into pv
                        raf = npool.tile([1, 512], dt.float32, tag="nf")
                        nc.vector.reciprocal_approx_fast(raf[:], da[:])
                        rr = npool.tile([1, 512], dt.float32, tag="nf")
                        nc.vector.tensor_mul(rr[:], raf[:], dv[:])
                        rr16 = npool.tile([1, 512], dt.bfloat16, tag="n16")
                        nc.scalar.copy(rr16[:], rr[:])
                        eas = psum1.tile([128, 512], dt.float32, tag="bc")
                        nc.tensor.matmul(
                            eas[:AL, :],
                            brow[0:1, 128 * (1 + h) : 128 * (1 + h) + AL],
                            rr16[:], start=True, stop=True,
                        )
                        ea2 = eapool.tile([AL, 512], dt.bfloat16, tag="ea2")
                        nc.vector.tensor_mul(ea2[:], ea[:], eas[:AL, :])
                        nc.tensor.matmul(
                            pv[:], avt[h][:], ea2[:], start=False, stop=True,
                            skip_group_check=True,
                        )
                        # normalize by 1/Dv and store attnT slice
                        rvf = npool.tile([1, 512], dt.float32, tag="nf")
                        nc.vector.reciprocal_approx_fast(rvf[:], dv[:])
                        rv16 = npool.tile([1, 512], dt.bfloat16, tag="n16")
                        nc.scalar.copy(rv16[:], rvf[:])
                        rvb_ps = psum1.tile([128, 512], dt.float32, tag="bc")
                        nc.tensor.matmul(
                            rvb_ps[:], brow[0:1, 0:128], rv16[:], start=True, stop=True
                        )
                        rvb = npool.tile([128, 512], dt.bfloat16, tag="rvb")
                        nc.scalar.copy(rvb[:], rvb_ps[:])
                        nc.vector.tensor_mul(
                            attnT[h][:, S * b + 512 * qc : S * b + 512 * (qc + 1)],
                            pv[:], rvb[:],
                        )
                # after batch b: bounce + AllGather
                for h in range(HPC):
                    nc.sync.dma_start(
                        bnc[b][128 * h : 128 * (h + 1), :], attnT[h][:, S * b : S * (b + 1)]
                    )
                nc.gpsimd.collective_compute(
                    "AllGather",
                    bass.mybir.AluOpType.bypass,
                    replica_groups=RG,
                    ins=[bnc[b][:, :].opt()],
                    outs=[agd[b][:, :].opt()],
                )



            # warmup collective: absorb ncfw/channel startup cost during load
            nc.gpsimd.collective_compute(
                "AllGather", bass.mybir.AluOpType.bypass, replica_groups=RG,
                ins=[wupin[:, :].opt()], outs=[wupout[:, :].opt()],
            )

            with tc.tile_pool(name="p1", bufs=1) as p1pool, tc.tile_pool(name="rope", bufs=2) as rp:
                wq_k = [p1pool.tile([128, 6 * HD], dt.bfloat16, tag=f"wq{k}", name=f"wq{k}") for k in range(KT16)]
                # xa tiles per (cchunk, k): col ranges [0:522),[522:1034),[1034:1546),[1546:2058)
                ccol = [(0, 522), (522, 512), (1034, 512), (1546, 512)]
                xs = [
                    [p1pool.tile([128, 522], dt.bfloat16, tag=f"xa{min(ci,3) if ci < 3 else 0}_{k}", name=f"xa{ci}_{k}") for k in range(KT16)]
                    for ci in range(4)
                ]
                for k in range(KT16):
                    nc.sync.dma_start(wq_k[k][:, 256:512], wqkv[128 * k : 128 * (k + 1), 256:512])
                    x0, xw = ccol[0]
                    nc.sync.dma_start(xs[0][k][:, :xw], xa[128 * k : 128 * (k + 1), x0 : x0 + xw])
                for k in range(KT16):
                    nc.sync.dma_start(wq_k[k][:, 0:256], wqkv[128 * k : 128 * (k + 1), 0:256])
                    nc.sync.dma_start(wq_k[k][:, 512:768], wqkv[128 * k : 128 * (k + 1), 512:768])
                # consts after the chunk-0-critical loads, before the rest of xa
                nc.sync.dma_start(c2[:], c2d[:, :])
                nc.sync.dma_start(s2[:], s2d[:, :])
                nc.sync.dma_start(ident[:], identd[:, :])
                nc.sync.dma_start(tri[:], trid[:, :])
                nc.sync.dma_start(g2m[:], g2md[:, :])
                nc.sync.dma_start(brow[:], browd[:, :])
                nc.sync.dma_start(wo_sb[:], wo[:, :].rearrange("(k p) c -> p k c", p=128))
                for ci in range(1, 4):
                    x0, xw = ccol[ci]
                    for k in range(KT16):
                        nc.sync.dma_start(xs[ci][k][:, :xw], xa[128 * k : 128 * (k + 1), x0 : x0 + xw])


                def proj_group(m, c0, w, row, xoff):
                    ps = psum3.tile([128, 512], dt.float32, tag="mm")
                    for k in range(KT16):
                        nc.tensor.matmul(
                            ps[:, :w],
                            wq_k[k][:, 128 * m : 128 * (m + 1)],
                            row[k][:, xoff : xoff + w],
                            start=(k == 0), stop=(k == KT16 - 1),
                        )
                    nc.scalar.copy(pdst[m][:, c0 : c0 + w], ps[:, :w])


                def rope_chunk(xr, xi, tc0, c0):
                    # tc0: token col offset in [0,2048); c0 = AL + tc0 (col in pdst)
                    cs = c2[:, tc0 : tc0 + 512]
                    sn = s2[:, tc0 : tc0 + 512]
                    a = rp.tile([128, 512], dt.bfloat16, tag="ra")
                    b_ = rp.tile([128, 512], dt.bfloat16, tag="rb")
                    nc.vector.tensor_mul(a[:], xr[:, c0 : c0 + 512], cs)
                    nc.vector.tensor_mul(b_[:], xi[:, c0 : c0 + 512], sn)
                    ro = rp.tile([128, 512], dt.bfloat16, tag="rro")
                    nc.vector.tensor_sub(ro[:], a[:], b_[:])
                    c_ = rp.tile([128, 512], dt.bfloat16, tag="rc")
                    d_ = rp.tile([128, 512], dt.bfloat16, tag="rd")
                    nc.vector.tensor_mul(c_[:], xr[:, c0 : c0 + 512], sn)
                    nc.vector.tensor_mul(d_[:], xi[:, c0 : c0 + 512], cs)
                    io = rp.tile([128, 512], dt.bfloat16, tag="rio")
                    nc.vector.tensor_add(io[:], c_[:], d_[:])
                    return ro, io

                def post_m(m, ci):
                    c0 = AL + 512 * ci
                    tc0 = 512 * ci
                    if m == 3:   # KR+KI done for this chunk
                        ro, io = rope_chunk(KR, KI, tc0, c0)
                        for h in range(HPC):
                            hs = slice(64 * h, 64 * h + 64)
                            nc.sync.dma_start(KTt[h][0:64, c0 : c0 + 512], ro[hs, :])
                            nc.sync.dma_start(KTt[h][64:128, c0 : c0 + 512], io[hs, :])
                    elif m == 1:  # QR+QI done
                        ro, io = rope_chunk(QR, QI, tc0, c0)
                        for h in range(HPC):
                            hs = slice(64 * h, 64 * h + 64)
                            nc.sync.dma_start(QT[h][0:64, c0 : c0 + 512], ro[hs, :])
                            nc.sync.dma_start(QT[h][64:128, c0 : c0 + 512], io[hs, :])
                    elif m >= 4:  # V chunk ready -> transposes
                        h = m - 4
                        bb, thalf = ci // 2, 4 * (ci % 2)
                        for tt in range(4):
                            tp = psum.tile([128, 128], dt.bfloat16, tag="pv")
                            nc.tensor.transpose(tp[:], VT[h][:, c0 + 128 * tt : c0 + 128 * (tt + 1)], ident[:])
                            nc.scalar.copy(vtr[h][:, 8 * bb + thalf + tt, :], tp[:])

                # chunk 0 solo (starts as soon as xs[0] lands), then pairs (1,2), then 3
                for m in (2, 3, 0, 1, 4, 5):
                    psa = psum3.tile([128, 512], dt.float32, tag="mm")
                    for k in range(KT16):
                        nc.tensor.matmul(psa[:], wq_k[k][:, 128 * m : 128 * (m + 1)],
                                         xs[0][k][:, AL : AL + 512],
                                         start=(k == 0), stop=(k == KT16 - 1))
                    nc.scalar.copy(pdst[m][:, AL : AL + 512], psa[:])
                    post_m(m, 0)
                # adapter column groups (K and V only) — emitted after chunk 0 so the
                # LDW-bound tiny matmuls run on a warm PE instead of gating the start
                for m in (2, 3, 4, 5):
                    proj_group(m, 0, AL, xs[0], 0)
                for h in range(HPC):
                    hs = slice(64 * h, 64 * h + 64)
                    nc.sync.dma_start(KTt[h][0:64, 0:AL], KR[hs, 0:AL])
                    nc.sync.dma_start(KTt[h][64:128, 0:AL], KI[hs, 0:AL])
                    tp = psum.tile([128, 128], dt.bfloat16, tag="pv")
                    nc.tensor.transpose(tp[:AL, :], VT[h][:, 0:AL], ident[:])
                    nc.scalar.copy(avt[h][:], tp[:AL, :])
                for ca, cb in ((1, 2),):
                    for m in (2, 3, 0, 1, 4, 5):
                        psa = psum3.tile([128, 512], dt.float32, tag="mm")
                        psb = psum3.tile([128, 512], dt.float32, tag="mm")
                        for k in range(KT16):
                            lhs = wq_k[k][:, 128 * m : 128 * (m + 1)]
                            nc.tensor.matmul(psa[:], lhs, xs[ca][k][:, 0:512],
                                             start=(k == 0), stop=(k == KT16 - 1))
                            nc.tensor.matmul(psb[:], lhs, xs[cb][k][:, 0:512],
                                             start=(k == 0), stop=(k == KT16 - 1))
                        nc.scalar.copy(pdst[m][:, AL + 512 * ca : AL + 512 * (ca + 1)], psa[:])
                        nc.scalar.copy(pdst[m][:, AL + 512 * cb : AL + 512 * (cb + 1)], psb[:])
                        post_m(m, ca)
                        post_m(m, cb)
                attn_batch(0)
                for m in (2, 3, 0, 1, 4, 5):
                    psa = psum3.tile([128, 512], dt.float32, tag="mm")
                    for k in range(KT16):
                        nc.tensor.matmul(psa[:], wq_k[k][:, 128 * m : 128 * (m + 1)],
                                         xs[3][k][:, 0:512],
                                         start=(k == 0), stop=(k == KT16 - 1))
                    nc.scalar.copy(pdst[m][:, AL + 512 * 3 : AL + 512 * 4], psa[:])
                    post_m(m, 3)
                attn_batch(1)

            # ---- phase 5: o_proj on gathered heads ----
            ogp = ctx.enter_context(tc.tile_pool(name="ogp", bufs=2))
            for b in range(B):
                ag_k = [ogp.tile([128, S], dt.bfloat16, tag=f"ag{k}", name=f"ag{k}") for k in range(KT16)]
                for k in range(KT16):
                    nc.sync.dma_start(ag_k[k][:], agd[b][128 * k : 128 * (k + 1), :])
                for j in range(HPC):
                    pa = psum3.tile([128, 512], dt.float32, tag="mm")
                    pb = psum3.tile([128, 512], dt.float32, tag="mm")
                    for k in range(KT16):
                        lhs = wo_sb[:, k, 128 * j : 128 * (j + 1)]
                        nc.tensor.matmul(pa[:], lhs, ag_k[k][:, 0:512],
                                         start=(k == 0), stop=(k == KT16 - 1))
                        nc.tensor.matmul(pb[:], lhs, ag_k[k][:, 512:1024],
                                         start=(k == 0), stop=(k == KT16 - 1))
                    for qc, ps in ((0, pa), (1, pb)):
                        osb = ogp.tile([128, 512], dt.float32, tag="osb")
                        nc.scalar.copy(osb[:], ps[:])
                        nc.sync.dma_start(
                            out_ext[128 * j : 128 * (j + 1),
                                    S * b + 512 * qc : S * b + 512 * (qc + 1)],
                            osb[:],
                        )

    nc.finalize()
    return nc


def _host_prep(inputs):
    """Build the 8 per-core input maps from full inputs."""
    x = np.asarray(inputs["x"], np.float32)
    adapter = np.asarray(inputs["adapter"], np.float32)
    wq = np.asarray(inputs["wq"], np.float32)
    wk = np.asarray(inputs["wk"], np.float32)
    wv = np.asarray(inputs["wv"], np.float32)
    wo = np.asarray(inputs["wo"], np.float32)
    g1 = np.asarray(inputs["gate1"], np.float32).reshape(H)
    g2 = np.asarray(inputs["gate2"], np.float32).reshape(H)
    fc = np.asarray(inputs["freqs_cos"], np.float32)  # [S, 64]
    fs = np.asarray(inputs["freqs_sin"], np.float32)
    vs = int(inputs["video_start"])
    assert vs + MF <= 128, "gate2 block must stay in kt tile 0"

    # xa: [D, 10+2048] = adapter^T ++ x^T (bf16)
    xt = x.reshape(TOK, D).T
    at = adapter.reshape(AL, D).T
    xa = np.concatenate([at, xt], axis=1).astype(BF16)

    # RoPE split permutation per head: even dims then odd dims
    ev = np.arange(0, HD, 2)
    od = np.arange(1, HD, 2)

    # c2/s2: [128, 2048]; rows 0-63 for head h0's pairs, 64-127 for h1's pairs
    cosT = np.tile(fc.T, (1, B))  # [64, 2048]
    sinT = np.tile(fs.T, (1, B))
    c2 = np.vstack([cosT, cosT]).astype(BF16)
    s2 = np.vstack([sinT, sinT]).astype(BF16)

    tri = np.triu(np.ones((HD, HD), np.float32)).astype(BF16)
    ident = np.eye(HD, dtype=np.float32).astype(BF16)

    in_maps = []
    for c in range(NCORES):
        hs = [HPC * c + i for i in range(HPC)]  # global head ids
        # paired-head m-tiles: QR=[h0_even,h1_even], QI=[h0_odd,h1_odd], same for K; V=[h0],[h1]
        def rows(w, h):  # weight rows for head h -> [128, D]
            return w[HD * h : HD * (h + 1), :]

        qr = np.vstack([rows(wq, hs[0])[ev], rows(wq, hs[1])[ev]])
        qi = np.vstack([rows(wq, hs[0])[od], rows(wq, hs[1])[od]])
        kr = np.vstack([rows(wk, hs[0])[ev], rows(wk, hs[1])[ev]])
        ki = np.vstack([rows(wk, hs[0])[od], rows(wk, hs[1])[od]])
        v0 = rows(wv, hs[0])
        v1 = rows(wv, hs[1])
        wqkv = np.concatenate([m.T for m in (qr, qi, kr, ki, v0, v1)], axis=1).astype(BF16)

        woc = wo.T[:, HPC * HD * c : HPC * HD * (c + 1)].astype(BF16)  # [D, 256]

        g2mat = np.ones((HD, HPC * S), np.float32)
        for i, h in enumerate(hs):
            blk = np.ones((HD, S), np.float32)
            blk[vs : vs + MF, vs + MF :] = math.exp(g2[h])
            g2mat[:, S * i : S * (i + 1)] = blk
        g2mat = g2mat.astype(BF16)

        brow = np.zeros((1, 3 * HD), np.float32)
        brow[0, 0:HD] = 1.0
        for i, h in enumerate(hs):
            brow[0, HD * (1 + i) : HD * (2 + i)] = math.tanh(g1[h])
        brow = brow.astype(BF16)

        in_maps.append(
            {
                "xa": xa, "wqkv": wqkv, "wo": woc, "c2": c2, "s2": s2,
                "tri": tri, "ident": ident, "g2m": g2mat, "brow": brow,
            }
        )
    return in_maps


def _enable_ldw_opt():
    """Walrus dedups consecutive identical LDWEIGHTS only with ldw-opt on;
    our chunk-paired accumulation groups repeat each stationary operand."""
    from concourse import bass_utils
    if getattr(bass_utils, "_ldw_patched", False):
        return
    orig = bass_utils.run_command

    def patched(cmd, *a, **kw):
        if isinstance(cmd, list):
            cmd = [c.replace("--enable-ldw-opt=false", "--enable-ldw-opt=true") if isinstance(c, str) else c for c in cmd]
        return orig(cmd, *a, **kw)

    bass_utils.run_command = patched
    bass_utils._ldw_patched = True


def _ensure_ntff_hook():
    import sys, types
    if "antenv.axon_hooks" in sys.modules:
        return
    try:
        from trn_agent_boot.trn_boot import _ntff_profile_via_ctypes
        hook = _ntff_profile_via_ctypes("/opt/axon/libaxon_pjrt.so")
        mod = types.ModuleType("antenv.axon_hooks")
        mod.get_axon_ntff_profile_hook = lambda: hook
        mod.set_axon_ntff_profile_hook = lambda h: None
        sys.modules["antenv.axon_hooks"] = mod
    except Exception:
        pass


def kernel(**inputs):
    global _BUILT, LAST_EXEC_NS
    import os
    from concourse.bass_utils import run_bass_kernel_spmd

    if _BUILT is None:
        _BUILT = _build()
    nc = _BUILT
    in_maps = _host_prep(inputs)
    trace = bool(os.environ.get("KERNEL_TRACE"))
    if trace:
        _ensure_ntff_hook()
    res = run_bass_kernel_spmd(
        nc, in_maps, core_ids=list(range(NCORES)), trace=trace
    )
    LAST_EXEC_NS = res.exec_time_ns
    outs = [np.asarray(r["out"], np.float32) for r in res.results]
    # out_c: [256, 2048] = out^T[j_local, b*1024+s] -> full [B, S, D]
    full = np.concatenate(
        [o.reshape(HPC * HD, B, S).transpose(1, 2, 0) for o in outs], axis=2
    )
    return full.astype(np.float32)
